# revision 3
# baseline (speedup 1.0000x reference)
"""Neural CDE Trainium2 kernel — v3 (f0y-space scan, shortest chain).

Strategy: pure data parallelism over batch B=128 -> 8 cores x 16 rows.
Per core, the T-1=1023-step RK4 scan runs fully unrolled. Layout:
activations [feature_on_partition, batch_on_free].

v3 core idea: never materialize y. Track f0y := fw0 @ y [W=128, BS].
Each stage j's layer-1 pre-activation X_j = fw0 @ x_j is assembled in
PSUM as:
    X_{j+1} = I@f0y + korr2x(dxs) + W_c @ rZ_j     (c = 0..3)
where W_c[p, w] = fw0[w, 16c + p%16] folds the G-selector reduction and
the fw0 matmul of the *next* layer-1 into one PE group, and
korr2x[d, w] = a_j * sum_h fw0[w, h] carries the a*S tanh-correction term.
The serial chain per stage is just:
    (W-mms) -> Exp -> Ln -> p2 -> Exp -> Ln -> p3 x4 -> Exp -> +1 ->
    recip -> mult -> (W-mms of next stage)
f0y itself advances once per step entirely OFF the chain via DVE:
    acc = sum_j cj * X_{j+1};  f0y' = acc - (4/3) * f0y
(since sum_j cj = 7/3 and X_{j+1} = f0y + fw0@kps_j).
The readout needs lw @ y_T = (lw @ pinv(fw0)) @ f0y_T — one constant.

Math (as v2): softplus = Ln(Exp(z)+1) via pinned natural_log_exp table;
tanh(v) = 1 - 2/(1+exp(2v)); -2*a_j folded into the zb selector.
fw2 rows permuted so chunk c / partition p hold (h = 16c + p%16, d = p//16).
"""

import numpy as np

B, T, D, H, W = 128, 1024, 8, 64, 128
NCORES = 8
BS = B // NCORES          # 16 batch rows per core
NSTEPS_FULL = T - 1       # 1023

_CJ = (1.0 / 3.0, 2.0 / 3.0, 1.0 / 3.0, 1.0)  # u_j / alpha_j
_SROW = (0, 1, 1, 2)                   # dX variant per stage
_AVARIANT = (0, 0, 1, 2)               # alpha variant {0.5, 1.0, 1/6}
_AVALS = (0.5, 1.0, 1.0 / 6.0)
# korr2x scale per stage = alpha_j; unique values {0.5, 1.0, 1/6}
_K2VARIANT = (0, 0, 1, 2)

# wconst free-dim layout: name -> (partitions, free_offset, free_len)
_L = {}
_off = 0
for _name, _p, _f in [
    ("fw0p", H, W), ("fw1p", W, W), ("fw2p", W, 512),
    ("wf0", 128, 4 * W), ("i128", 128, 128), ("f0rs", D, 3 * W),
    ("ebcn", D, 3 * 128),
    ("b3l", 4, 128), ("b3r", 4, 4 * BS),
    ("iw0p", D, W), ("iw1p", W, W), ("iw2p", W, H),
    ("x0T", D, BS), ("lpT", W, 1),
    ("ib0", W, 1), ("ib1", W, 1), ("ib2", H, 1),
    ("fb0", W, 1), ("fb1", W, 1), ("lbneg", 1, 1),
]:
    _L[_name] = (_p, _off, _f)
    _off += _f
WCONST_F = _off


def _hd_orig(c, p):
    h = 16 * c + (p % 16)
    d = p // 16
    return h * D + d


def build_bass(nsteps):
    import concourse.bass as bass
    import concourse.bacc as bacc
    import concourse.mybir as mybir
    from concourse import tile

    f32 = mybir.dt.float32
    AF = mybir.ActivationFunctionType
    ALU = mybir.AluOpType

    class _PinnedActBacc(bacc.Bacc):
        """Only offer natural_log_exp_and_others to the act-table placement
        pass (covers Exp/Ln/Relu/Identity/Copy). The default greedy placement
        alternates exp_and_others <-> natural_log, costing a 1283ns table load
        at every Exp<->Ln transition (4x per RK4 stage)."""

        _PIN = "natural_log_exp_and_others"

        def insert_act_table_loads(self):
            import bass_rust as _bass_rust
            from concourse.hw_specs import get_activation_tables

            has_activation = any(
                isinstance(i, mybir.InstActivation)
                for b in self.main_func.blocks
                for i in b.instructions
            )
            if not has_activation:
                return
            tables = [
                (n, (funcs if n == self._PIN else set()))
                for n, funcs in get_activation_tables(self.m.arch).items()
            ]
            assert any(funcs for _, funcs in tables), "pinned act table missing"
            _bass_rust.insert_act_table_loads(self, tables)

    nc = _PinnedActBacc(None)

    wc_d = nc.declare_dram_parameter("wconst", [128, WCONST_F], f32, isOutput=False)
    dxt_d = [
        nc.declare_dram_parameter(f"dxt{s}", [D, nsteps * BS], f32, isOutput=False)
        for s in range(3)
    ]
    out_d = nc.declare_dram_parameter("out", [1, BS], f32, isOutput=True)

    with tile.TileContext(nc) as tc:
        with (
            tc.tile_pool(name="const", bufs=1) as cpool,
            tc.tile_pool(name="f0y", bufs=2) as fpool,
            tc.tile_pool(name="facc", bufs=1) as accpool,
            tc.tile_pool(name="work16", bufs=2) as w16,
            tc.tile_pool(name="work64", bufs=2) as w64,
            tc.tile_pool(name="ps_x", bufs=2, space="PSUM") as ps_x,
            tc.tile_pool(name="ps_p2", bufs=1, space="PSUM") as ps_p2,
            tc.tile_pool(name="ps_u", bufs=1, space="PSUM") as ps_u,
            tc.tile_pool(name="ps_zb", bufs=1, space="PSUM") as ps_zb,
            tc.tile_pool(name="ps_p3", bufs=1, space="PSUM") as ps_p3,
        ):
            wc = cpool.tile([128, WCONST_F], f32, tag="wconst")
            nc.sync.dma_start(wc[:], wc_d[:])
            dxt = []
            for s in range(3):
                dt_ = cpool.tile([D, nsteps * BS], f32, tag=f"dxt{s}")
                nc.sync.dma_start(dt_[:], dxt_d[s][:])
                dxt.append(dt_)

            def C(name):
                p, o, f = _L[name]
                return wc[0:p, o : o + f]

            # Warm each non-PE engine's vector clock on the const DMAs.
            warm = w16.tile([1, 4], f32, tag="warm")
            nc.scalar.activation(warm[0:1, 0:1], wc[0:1, 0:1], AF.Copy)
            nc.vector.tensor_copy(warm[0:1, 1:2], wc[0:1, 0:1])
            for s in range(3):
                nc.vector.tensor_copy(warm[0:1, 1:2], dxt[s][0:1, 0:1])

            # ---- y0 = init_mlp(x0); X_0 = fw0 @ y0 ----
            yinit = w16.tile([H, BS], f32, tag="yinit")
            pi = ps_x.tile([W, BS], f32, tag="x")
            nc.tensor.matmul(pi[:], C("iw0p"), C("x0T"), start=True, stop=True)
            h1 = w16.tile([W, BS], f32, tag="s")
            nc.scalar.activation(h1[:], pi[:], AF.Relu, bias=C("ib0"))
            pi2 = ps_p2.tile([W, BS], f32, tag="p2")
            nc.tensor.matmul(pi2[:], C("iw1p"), h1[:], start=True, stop=True)
            h2 = w16.tile([W, BS], f32, tag="s")
            nc.scalar.activation(h2[:], pi2[:], AF.Relu, bias=C("ib1"))
            pk = ps_p3.tile([H, BS], f32, tag="p3")
            nc.tensor.matmul(pk[:], C("iw2p"), h2[:], start=True, stop=True)
            nc.scalar.activation(yinit[:], pk[:], AF.Identity, bias=C("ib2"))

            # f0y = fw0 @ y0 (via psum + DVE copy)
            Xi = ps_x.tile([W, BS], f32, tag="x")
            nc.tensor.matmul(Xi[:], C("fw0p"), yinit[:], start=True, stop=True)
            f0y = fpool.tile([W, BS], f32, tag="f0y")
            nc.vector.tensor_copy(f0y[:], Xi[:])
            acc = accpool.tile([W, BS], f32, tag="acc")
            X = None  # stage input psum; j==0 reads f0y (SBUF) instead
            pending_acc = None

            # ---- the scan ----
            for t in range(nsteps):
                tb = t * BS
                for j in range(4):
                    s = _SROW[j]
                    av = _AVARIANT[j]
                    cj = _CJ[j]
                    dxs = dxt[s][:, tb : tb + BS]

                    # PE (off-chain): assemble next X start + selectors
                    Xn = ps_x.tile([W, BS], f32, tag="x")
                    nc.tensor.matmul(Xn[:], C("i128"), f0y[:], start=True, stop=False)
                    k2 = C("f0rs")
                    kv = _K2VARIANT[j]
                    nc.tensor.matmul(
                        Xn[:], k2[:, kv * W : (kv + 1) * W], dxs,
                        start=False, stop=False,
                    )
                    zb = ps_zb.tile([128, BS], f32, tag="zb")
                    nc.tensor.matmul(
                        zb[:], C("ebcn")[:, av * 128 : (av + 1) * 128], dxs,
                        start=True, stop=True,
                    )

                    # chain: softplus layer 1 (j=0 reads f0y SBUF, else X_j psum)
                    xin = f0y if j == 0 else X
                    u1 = ps_u.tile([W, BS], f32, tag="u")
                    nc.scalar.activation(u1[:], xin[:], AF.Exp, bias=C("fb0"))

                    # deferred acc += c_{j-1} * X_j — issued AFTER Exp-u1 so the
                    # wait optimizer doesn't route Exp's X-psum dep through this
                    # DVE op (which would put it on the chain).
                    if pending_acc is not None:
                        pj, pX = pending_acc
                        if pj == 0:
                            nc.vector.tensor_scalar_mul(acc[:], pX[:], _CJ[0])
                        else:
                            nc.vector.affine_then_add(acc[:], pX[:], acc[:], _CJ[pj], 0.0)
                        pending_acc = None
                    s1 = w16.tile([W, BS], f32, tag="s")
                    nc.scalar.activation(s1[:], u1[:], AF.Ln, bias=1.0)

                    # chain: layer 2
                    p2 = ps_p2.tile([W, BS], f32, tag="p2")
                    nc.tensor.matmul(p2[:], C("fw1p"), s1[:], start=True, stop=True)

                    # off-chain: fb2 bias into p3 (rank-4 mm)
                    p3 = ps_p3.tile([128, 4 * BS], f32, tag="p3")
                    nc.tensor.matmul(p3[:], C("b3l"), C("b3r"), start=True, stop=False)

                    u2 = ps_u.tile([W, BS], f32, tag="u")
                    nc.scalar.activation(u2[:], p2[:], AF.Exp, bias=C("fb1"))
                    s2 = w16.tile([W, BS], f32, tag="s")
                    nc.scalar.activation(s2[:], u2[:], AF.Ln, bias=1.0)

                    # chain: layer 3 (4 chunks)
                    fw2p = C("fw2p")
                    for c in range(4):
                        nc.tensor.matmul(
                            p3[:, c * BS : (c + 1) * BS],
                            fw2p[:, c * 128 : (c + 1) * 128],
                            s2[:],
                            start=False, stop=(c == 3),
                        )

                    # chain: tanh tail
                    texp = w64.tile([128, 4 * BS], f32, tag="texp")
                    nc.scalar.activation(texp[:], p3[:], AF.Exp, scale=2.0)
                    den = w64.tile([128, 4 * BS], f32, tag="den")
                    nc.vector.tensor_scalar(
                        den[:], texp[:], 1.0e30, 1.0, ALU.min, ALU.add
                    )
                    r = w64.tile([128, 4 * BS], f32, tag="r")
                    nc.vector.reciprocal_approx_fast(r[:], den[:])
                    rZ = w64.tile([128, 4, BS], f32, tag="rZ")
                    zb_b = zb[:, :]
                    zb_b = bass.AP(
                        zb_b.tensor, zb_b.offset,
                        [zb_b.ap[0], [0, 4], zb_b.ap[1]],
                    )
                    r3 = r[:, :]
                    r3 = bass.AP(
                        r3.tensor, r3.offset,
                        [r3.ap[0], [BS, 4], [1, BS]],
                    )
                    nc.vector.tensor_tensor(rZ[:], r3, zb_b, ALU.mult)

                    # chain: W-mms complete X_{j+1} = fw0 @ x_{j+1}
                    wf0 = C("wf0")
                    for c in range(4):
                        nc.tensor.matmul(
                            Xn[:],
                            wf0[:, c * W : (c + 1) * W],
                            rZ[:, c, :],
                            start=False, stop=(c == 3),
                        )

                    # off-chain DVE: acc += cj * X_{j+1}. For j<3, defer past the
                    # next stage's Exp-u1 (see above). At j=3 the f0y update IS
                    # the next stage's input, so it must be issued now.
                    if j == 3:
                        nc.vector.affine_then_add(acc[:], Xn[:], acc[:], cj, 0.0)
                        # f0y' = acc - (4/3) f0y   (sum cj = 7/3)
                        f0y_new = fpool.tile([W, BS], f32, tag="f0y")
                        nc.vector.affine_then_add(
                            f0y_new[:], f0y[:], acc[:], -4.0 / 3.0, 0.0
                        )
                        f0y = f0y_new
                    else:
                        pending_acc = (j, Xn)

                    X = Xn

            # ---- readout: sigmoid((lw @ pinv(fw0)) @ f0y_T + lb) ----
            pr = ps_zb.tile([1, BS], f32, tag="zb")
            nc.tensor.matmul(pr[:], C("lpT"), f0y[:], start=True, stop=True)
            er = w16.tile([1, BS], f32, tag="er")
            nc.scalar.activation(er[:], pr[:], AF.Exp, bias=C("lbneg"), scale=-1.0)
            dr = w16.tile([1, BS], f32, tag="dr")
            nc.vector.tensor_scalar_add(dr[:], er[:], 1.0)
            rr = w16.tile([1, BS], f32, tag="rr")
            nc.vector.reciprocal(rr[:], dr[:])
            nc.sync.dma_start(out_d[:], rr[:])

    nc.compile()
    return nc


def prep_inputs(ts, coeff_d, coeff_c, coeff_b, coeff_a,
                iw0, ib0, iw1, ib1, iw2, ib2,
                fw0, fb0, fw1, fb1, fw2, fb2, lw, lb, nsteps=NSTEPS_FULL):
    """Build per-core input maps (host-side numpy prep)."""
    f = np.float32
    cd = np.asarray(coeff_d, f)[:, :nsteps, :]
    cc = np.asarray(coeff_c, f)[:, :nsteps, :]
    cb = np.asarray(coeff_b, f)[:, :nsteps, :]
    ca = np.asarray(coeff_a, f)

    dX1 = cb
    dX23 = 0.75 * cd + cc + cb
    dX4 = 3.0 * cd + 2.0 * cc + cb

    fw0 = np.asarray(fw0, f)
    fw2 = np.asarray(fw2, f)
    fb2 = np.asarray(fb2, f)

    def fill(wc, name, arr):
        p, o, fl = _L[name]
        assert arr.shape == (p, fl), (name, arr.shape, (p, fl))
        wc[0:p, o : o + fl] = arr

    wc0 = np.zeros((128, WCONST_F), f)
    fill(wc0, "fw0p", np.ascontiguousarray(fw0.T))
    fill(wc0, "fw1p", np.ascontiguousarray(np.asarray(fw1, f).T))
    fw2p = np.zeros((W, 512), f)
    b3l = np.zeros((4, 128), f)
    for c in range(4):
        for p in range(128):
            hd = _hd_orig(c, p)
            fw2p[:, c * 128 + p] = fw2[hd, :]
            b3l[c, p] = fb2[hd]
    fill(wc0, "fw2p", fw2p)
    fill(wc0, "b3l", b3l)
    b3r = np.zeros((4, 4 * BS), f)
    for c in range(4):
        b3r[c, c * BS : (c + 1) * BS] = 1.0
    fill(wc0, "b3r", b3r)
    # W_c[p, c*W + w] = fw0[w, 16c + p%16]
    wf0 = np.zeros((128, 4 * W), f)
    for c in range(4):
        for p in range(128):
            wf0[p, c * W : (c + 1) * W] = fw0[:, 16 * c + (p % 16)]
    fill(wc0, "wf0", wf0)
    fill(wc0, "i128", np.eye(128, dtype=f))
    # f0rs[d, v*W + w] = scale_v * sum_h fw0[w, h]
    f0r = fw0.sum(axis=1)  # [W]
    f0rs = np.zeros((D, 3 * W), f)
    for vi, sv in enumerate(_AVALS):
        f0rs[:, vi * W : (vi + 1) * W] = sv * f0r[None, :]
    fill(wc0, "f0rs", f0rs)
    ebcn = np.zeros((D, 3 * 128), f)
    for ai, aval in enumerate(_AVALS):
        for p in range(128):
            ebcn[p // 16, ai * 128 + p] = -2.0 * aval
    fill(wc0, "ebcn", ebcn)
    fill(wc0, "iw0p", np.ascontiguousarray(np.asarray(iw0, f).T))
    fill(wc0, "iw1p", np.ascontiguousarray(np.asarray(iw1, f).T))
    fill(wc0, "iw2p", np.ascontiguousarray(np.asarray(iw2, f).T))
    # lp = lw @ pinv(fw0): [1, W]
    lp = (np.asarray(lw, np.float64).reshape(1, H)
          @ np.linalg.pinv(fw0.astype(np.float64))).astype(f)
    fill(wc0, "lpT", np.ascontiguousarray(lp.T))
    fill(wc0, "ib0", np.asarray(ib0, f)[:, None])
    fill(wc0, "ib1", np.asarray(ib1, f)[:, None])
    fill(wc0, "ib2", np.asarray(ib2, f)[:, None])
    fill(wc0, "fb0", np.asarray(fb0, f)[:, None])
    fill(wc0, "fb1", np.asarray(fb1, f)[:, None])
    fill(wc0, "lbneg", -np.asarray(lb, f).reshape(1, 1))

    in_maps = []
    for i in range(NCORES):
        sl = slice(i * BS, (i + 1) * BS)
        wc = wc0.copy()
        fill(wc, "x0T", np.ascontiguousarray(ca[sl, 0, :].T))
        m = {"wconst": wc}
        for name, arr in (("dxt0", dX1), ("dxt1", dX23), ("dxt2", dX4)):
            m[name] = np.ascontiguousarray(
                arr[sl].transpose(2, 1, 0).reshape(D, -1)
            )
        in_maps.append(m)
    return in_maps


_CACHE = {}


def _get_nc(nsteps):
    if nsteps not in _CACHE:
        _CACHE[nsteps] = build_bass(nsteps)
    return _CACHE[nsteps]


def kernel(**inputs):
    from concourse.bass_utils import run_bass_kernel_spmd

    nsteps = NSTEPS_FULL
    in_maps = prep_inputs(nsteps=nsteps, **inputs)
    nc = _get_nc(nsteps)
    res = run_bass_kernel_spmd(nc, in_maps, list(range(NCORES)))
    outs = [res.results[i]["out"].reshape(BS) for i in range(NCORES)]
    return np.concatenate(outs, axis=0).astype(np.float32)
